# revision 45
# baseline (speedup 1.0000x reference)
"""Bass/Trainium2 kernel for the BindsNet LIF module.

Math (per timestep t, reference order):
    x   = s_in[t] @ w                      # [B, N], state-independent!
    v   = decay*(v - REST) + REST
    v  += where(refrac <= 0, x, 0)
    refrac = max(refrac - 1, 0)
    s   = v >= THRESH
    refrac = where(s, 5, refrac)
    v   = where(s, RESET, v)

FAST PATH — refractory-schedule certificate (~28 us HW, 17.6x the dense
kernel below).  In u = v - REST coordinates the threshold is 13 while
x ~ 102 +- 1.5 with ALL matmul terms non-negative.  If x >= 13 at every
non-refractory step, every neuron spikes at t = 0, 6, 12, ... (period
REFRAC+1) and sits at u = 0 in between, so the output is exactly the
periodic pattern s[t] = (t % 6 == 0) — independent of the x values.  The
device therefore only has to PROVE x >= 13 at the 86 active steps: it
computes a certified lower bound (partial k-sum against per-8-neuron-group
column minima of w — both below x since all terms are >= 0), min-reduces
it, and the host checks min >= TAU.  TAU = 15.5 is rigorous against
worst-case fp8e4 rounding inflation (<= 1.129x + 0.31); the measured
device bound is 16.91.  That is ~1/48 the dense matmul work: 22K PE
cycles/core with 3 MB of fp8 DMA, wrapped in a p-state warmup chain and a
two-ring chunked DMA schedule tuned from perfetto traces.  If the check
fails (different data distribution / shapes), kernel() falls back to the
exact dense kernel below, or to a numpy simulation for untiled shapes —
correct for any input.

DENSE FALLBACK (the previous 493 us kernel, kept intact):
  phase A: XT = x.T via fp8 DoubleRow matmuls (spike-exact: worst-case
           |dx| < 13 vs margin 82), phase B: sequential LIF as ONE fused
           custom DVE op per timestep; host decodes spikes as (y == -5).

Distribution: data-parallel over batch.  B=32 -> 4 rows per core on 8
NeuronCores; weights (group-minima on the fast path) replicated; no
collectives.  Measured relative error 0.0 vs the f32 jax reference on
both paths.
"""

import os
import sys

import numpy as np

for _p in ("/opt/trn_rl_repo", "/root/.axon_site/_ro/trn_rl_repo"):
    if os.path.isdir(_p) and _p not in sys.path:
        sys.path.append(_p)

import ml_dtypes  # noqa: E402

import concourse.bacc as bacc  # noqa: E402
import concourse.mybir as mybir  # noqa: E402
import concourse.tile as tile  # noqa: E402
from concourse import dve_ops as _dve_ops  # noqa: E402
from concourse.dve_spec import (  # noqa: E402
    C0, C1, One, Spec, Src0, Src1, Zero, lower, select, sq,
    _has_src1 as has_src1,
)
from concourse.dve_table_gen import dve_ver_for  # noqa: E402
from concourse.dve_uop import DveOpSpec  # noqa: E402

P = 128  # partitions
TGS = 16  # timesteps per spike-DMA group
N_CORES = 8

# BindsNet LIFNodes constants (f32 exact)
DECAY = float(np.exp(np.float32(-1.0) / np.float32(100.0), dtype=np.float32))
U_THRESH = 13.0  # THRESH - REST = -52 - (-65)


def _scales(T):
    """Per-timestep decay prescale sc_t = 0.5 * decay**(-t) (f32)."""
    return (0.5 * np.exp(np.arange(T) / 100.0)).astype(np.float32)


def _block_sizes(MFULL):
    """m-block sizes: 512s with a tapered [384, 128] tail. The tapering
    shortens the post-last-matmul LIF stretch; the extra weight pass for
    the small blocks streams on spare DMA-ring bandwidth."""
    if MFULL <= 512:
        return [MFULL]
    assert MFULL % 512 == 0
    return [512] * (MFULL // 512 - 1) + [384, 128]

BF16 = mybir.dt.bfloat16
FP8 = mybir.dt.float8e4
F32 = mybir.dt.float32

SPIKE_MARK = -5.0  # y written on spike: encodes reset + 5-step refractory


def _register_lif_op():
    """Fused LIF step as one custom DVE instruction.

    Works in decay-prescaled coordinates: the host scales input column t by
    sc_t = 0.5 * decay**(-t), so the open-state membrane update is a pure
    add (z_t = z_{t-1} + x'_t) and the spike test is z >= 13*sc_t, with the
    per-step threshold passed as scalar s0. Single state y per neuron:
    y >= 0 is the scaled membrane z (x >= 0 and reset-to-0 keep it
    non-negative); y < 0 is a refractory countdown. Spike writes y = -5;
    five gated steps increment y back toward 0; the step after that is open
    — identical timing to the reference's refrac counter. Spikes are
    decoded host-side as (y == -5), which is exact (-5 written verbatim).

        a  = y + x'
        y' = (y >= 0) ? (a >= thr_t ? -5 : a) : y + 1
    """
    name = "LIF_STEP_ANT"
    if name in _dve_ops._SUB_OPCODE_FOR_NAME:
        return next(op for op in _dve_ops.OPS if op.name == name)
    a = Src0 + Src1
    body = select(Src0 >= Zero, select(a >= C0, C1, a), Src0 + One)

    def _ref(in0, in1, s0, s1, imm2):
        f32 = np.float32
        in0 = np.asarray(in0, f32)
        in1 = np.asarray(in1, f32).reshape(in0.shape)
        av = (in0 + in1).astype(f32)
        inner = np.where(av >= f32(s0), f32(s1), av)
        return np.where(in0 >= 0.0, inner, (in0 + f32(1.0)).astype(f32)
                        ).astype(f32)

    spec = Spec(body=body, reference=_ref)
    opcode = max(_dve_ops._SUB_OPCODE_FOR_NAME.values()) + 1
    _dve_ops._SUB_OPCODE_FOR_NAME[name] = opcode
    shas = {}
    for ver in ("v3", "v4"):
        try:
            tmp = DveOpSpec(name=name, opcode=opcode,
                            uops=lower(spec, ver=ver),
                            rd1_en=has_src1(spec))
            shas[ver] = tmp.sha(ver)
        except Exception:
            pass
    op = _dve_ops.DveOp(name, spec, subdim=False, uops_sha=shas,
                        perf_en={"v3": True, "v4": True})
    _dve_ops.OPS.append(op)
    _dve_ops.CUSTOM_DVE_SPECS[name] = spec
    return op


def build_nc(T, B, N):
    """Build the SPMD per-core Bass program.

    T timesteps, B local batch, N neurons (square weight [N, N])."""
    LIF_OP = _register_lif_op()  # lazy: keeps the cert NEFF free of the
    JT = N // P          # n tiles (output rows of XT)  # custom DVE tables
    KT2 = N // (2 * P)   # contraction tiles (DoubleRow: 256 k per matmul)
    MFULL = T * B        # local matmul rows, m = t*B + b
    SZS = _block_sizes(MFULL)  # m-block sizes; tapered tail for a short
    MB = len(SZS)              # final LIF stretch after the last matmul
    MOFF = [sum(SZS[:i]) for i in range(MB + 1)]
    JQ = min(4, JT)      # j tiles per w DMA (512 cols)
    NJQ = JT // JQ

    assert all(sz % (B * TGS) == 0 for sz in SZS) and JT % JQ == 0
    assert N % (2 * P) == 0

    thr = (np.float32(U_THRESH) * _scales(T)).astype(np.float32)

    nc = bacc.Bacc("TRN2", target_bir_lowering=False, debug=False,
                   num_devices=N_CORES)
    # Host pre-blocks both operands so every DMA is a fully-contiguous
    # stream: logical k = k2*256 + 2*p + i (p = partition, i = DoubleRow
    # slot), input m-columns grouped by m-block, weight n-columns by jq.
    inT = nc.dram_tensor("inT", [N * MFULL], FP8, kind="ExternalInput")
    wb = nc.dram_tensor("wb", [NJQ, KT2, P, 2, JQ * P], FP8,
                        kind="ExternalInput")
    spk = nc.dram_tensor("spk", [T // TGS, P, TGS * JT * B], BF16,
                         kind="ExternalOutput")

    with tile.TileContext(nc) as tc:
        with (
            tc.tile_pool(name="inT_p", bufs=2) as inT_p,
            tc.tile_pool(name="w_p", bufs=4) as w_p,
            tc.tile_pool(name="xt_p", bufs=2) as xt_p,
            tc.tile_pool(name="ps_p", bufs=8, space="PSUM") as ps_p,
            tc.tile_pool(name="sacc_p", bufs=3) as sacc_p,
            tc.tile_pool(name="st_p", bufs=1) as st_p,
        ):
            y0 = st_p.tile([P, JT * B], BF16)
            nc.vector.memset(y0[:], 0.0)
            y_prev = y0[:]

            # HAM warmup: ~4.5us of dummy matmuls on a zeroed tile so the
            # PE clock is at 8/8 before the first weights land (the clock
            # gate needs ~3.4us of sustained activity; the first real
            # matmul otherwise runs the whole ramp at half clock).
            warm_src = st_p.tile([P, P], FP8)
            nc.vector.memset(warm_src[:], 0.0)
            warm_ps = ps_p.tile([P, P], F32, name="warm_ps", tag="ps")
            for _ in range(44):
                nc.tensor.matmul(warm_ps[:], warm_src[:], warm_src[:],
                                 start=True, stop=True)

            # inT DMAs ride the Activation HWDGE ring (weights keep the SP
            # ring to themselves), prefetched one m-block ahead so the PE
            # never waits at an mb boundary. Every DMA is a contiguous
            # 1MB stream thanks to the host-side blocking.
            inT_tiles = {}

            def load_inT(mb, parts=2):
                mf = SZS[mb]
                t_ = inT_p.tile([P, KT2, 2, mf], FP8, name="inT_sb",
                                tag="inT_sb")
                parts = min(parts, KT2)
                kp = KT2 // parts
                src = inT[N * MOFF[mb]:N * MOFF[mb + 1]].rearrange(
                    "(k p i m) -> p k i m", k=KT2, p=P, i=2)
                for h in range(parts):
                    nc.scalar.dma_start(
                        t_[:, h * kp:(h + 1) * kp, :, :],
                        src[:, h * kp:(h + 1) * kp])
                inT_tiles[mb] = t_

            load_inT(0, parts=8)
            for mb in range(MB):
                MF = SZS[mb]
                TB = MF // B
                # ---- phase A: XT[:, j*MF + m] = x[t, b, n].T for this m block
                inT_sb = inT_tiles.pop(mb)
                xt = xt_p.tile([P, JT * MF], BF16, name="xt", tag="xt")
                for jq in range(NJQ):
                    if jq == min(1, NJQ - 1) and mb + 1 < MB:
                        load_inT(mb + 1)
                    pss = [ps_p.tile([P, MF], F32, name="ps", tag="ps")
                           for _ in range(JQ)]
                    wq = w_p.tile([P, KT2, 2, JQ * P], FP8, name="wq")
                    wparts = min(8 if (mb == 0 and jq == 0) else 2, KT2)
                    kp = KT2 // wparts
                    for h in range(wparts):
                        nc.sync.dma_start(
                            wq[:, h * kp:(h + 1) * kp, :, :],
                            wb[jq, h * kp:(h + 1) * kp].rearrange(
                                "k p i n -> p k i n"))
                    for kt in range(KT2):
                        for jj in range(JQ):
                            nc.tensor.matmul(
                                pss[jj][:],
                                wq[:, kt, :, jj * P:(jj + 1) * P],
                                inT_sb[:, kt, :, :],
                                start=(kt == 0), stop=(kt == KT2 - 1),
                                perf_mode=mybir.MatmulPerfMode.DoubleRow)
                    for jj in range(JQ):
                        j = jq * JQ + jj
                        nc.scalar.copy(xt[:, j * MF:(j + 1) * MF], pss[jj][:])

                # ---- phase B: sequential LIF, one fused DVE op per step.
                # The y history is the output; host decodes s = (y == -5).
                xt3 = xt.rearrange("p (j m) -> p j m", j=JT)
                for tg in range(TB // TGS):
                    g = MOFF[mb] // (B * TGS) + tg
                    sacc = sacc_p.tile([P, TGS * JT * B], BF16, name="sacc")
                    for tl in range(TGS):
                        tmb = tg * TGS + tl
                        t = MOFF[mb] // B + tmb
                        x3 = xt3[:, :, tmb * B:(tmb + 1) * B]
                        y_slot = sacc[:, tl * JT * B:(tl + 1) * JT * B]
                        nc.vector._custom_dve(
                            LIF_OP, out=y_slot, in0=y_prev, in1=x3,
                            s0=float(thr[t]), s1=SPIKE_MARK)
                        y_prev = y_slot
                    nc.scalar.dma_start(spk[g], sacc[:])

    nc.compile()
    return nc


_CACHE = {}

# test.py hooks: extra kwargs for run_bass_kernel_spmd (e.g. trace=True) and
# the last BassKernelResults, for HW exec time reporting.
_RUN_KWARGS = {}
_LAST_RESULT = None
_LAST_MINS = None


def _get_nc(T, B, N):
    key = (T, B, N)
    if key not in _CACHE:
        _CACHE[key] = build_nc(T, B, N)
    return _CACHE[key]


# ---------------------------------------------------------------------------
# Fast path: refractory-schedule certificate.
#
# In rest-shifted coordinates u = v - REST, the LIF threshold is
# U_THRESH = 13 while x[t,b,n] = sum_k in[t,b,k] w[k,n] ~ 102 +- 1.5 here
# (all terms non-negative).  If x >= 13 at every step where a neuron is
# non-refractory, then by induction every neuron spikes at t = 0, 6, 12, ...
# (REFRAC+1 = 6) and is refractory-silent in between: u is exactly 0 when
# each add lands, so the spike test is exactly x >= 13, and gated steps sit
# at u = 0 < 13.  The output is then the exact periodic pattern
# s[t,b,n] = (t % 6 == 0), independent of the x values.
#
# Because all terms a*w are >= 0, any partial sum is a certified lower
# bound.  We group neurons in GROUPs of 8 and precompute
# gmin[k,g] = min_{n in group g} w[k,n]; then a[m] @ gmin[:,g] <= x[m,n]
# for every n in the group, and a partial k-sum of it still lower-bounds x.
# The device computes this bound ONLY at the 86 active steps over the
# first K_USE k-terms (~1/59 of the dense matmul work), min-reduces over
# rows, and returns the per-group minima.  The host certifies min >= TAU
# (see TAU below for the worst-case fp8 rounding algebra) and emits the
# pattern; on failure it falls back to the exact dense kernel.  On the
# reference inputs the fp8-simulated bound min is 15.575 (the device
# reproduces it bit-exactly) vs TAU 15.4, while the true full x min is
# 95.1 vs threshold 13 — the certificate has enormous physical margin.
# ---------------------------------------------------------------------------

PERIOD = 6        # REFRAC/DT + 1
GROUP = 8         # neurons per gmin group
GSCL = 16.0       # power-of-2 prescale keeping fp8(gmin) in normal range
# Certified-if->=: worst-case fp8e4 round-to-nearest inflation of the
# device bound is <= 1.1289x + 0.561 (relative normal error 1.0625 on each
# operand plus subnormal absolute dregs), so bound/GSCL >= 15.4 proves the
# true partial sum >= (15.4 - 0.561)/1.1289 = 13.14 > 13 = U_THRESH.
TAU = 15.4
CHUNKS = (4, 5, 2, 2)  # weight kt tiles per DMA chunk
IN_CHUNKS = (2, 7, 2, 2)  # input kt chunks: small first chunk — in0's
# transfer gates the first matmul directly, and its successor streams at
# warm queue cadence with no re-arm gap, so kt2+ stays covered.
WARM_MM = 34      # PE p-state warmup matmuls issued while DMAs stream
# The certificate only needs a partial k-sum (any partial sum lower-bounds
# the full one).  For the 4096-k case the first 3328 terms still clear TAU
# (fp8-simulated min 15.575), saving 19% of the DMA bytes and matmuls.
# For other sizes use every k.
K_USE = {4096: 3328}


def build_cert_nc(KT2, M, NPJ):
    """Bound matmul + min-reduce.  KT2 = N/256 contraction tiles (DoubleRow),
    M = local active rows (= n_active_steps * B_local), NPJ = N/GROUP/128
    output partition tiles."""
    NP = NPJ * P
    nc = bacc.Bacc("TRN2", target_bir_lowering=False, debug=False,
                   num_devices=N_CORES)
    # Both operands host-blocked partition-major so every DMA chunk is a
    # per-partition-contiguous stream: logical k = k2*256 + 2*p + i.
    inT = nc.dram_tensor("inT", [P, KT2, 2, M], FP8, kind="ExternalInput")
    wc = nc.dram_tensor("wc", [P, KT2, 2, NP], FP8, kind="ExternalInput")
    mins = nc.dram_tensor("mins", [P, NPJ], F32, kind="ExternalOutput")

    # chunk boundaries along kt, per operand
    def _offsets(plan):
        offs, o = [], 0
        for ch in plan:
            offs.append((o, o + ch))
            o += ch
        assert o == KT2
        return offs

    if sum(CHUNKS) == KT2:
        w_offs, in_offs = _offsets(CHUNKS), _offsets(IN_CHUNKS)
    else:  # non-default shape: first chunk small, rest ~6
        plan, rem = [min(4, KT2)], KT2 - min(4, KT2)
        while rem > 6:
            plan.append(6)
            rem -= 6
        if rem:
            plan.append(rem)
        w_offs = in_offs = _offsets(plan)

    with tile.TileContext(nc) as tc:
        with (
            tc.tile_pool(name="in_p", bufs=1) as in_p,
            tc.tile_pool(name="w_p", bufs=1) as w_p,
            tc.tile_pool(name="warm_p", bufs=1) as warm_p,
            tc.tile_pool(name="ps_p", bufs=NPJ, space="PSUM") as ps_p,
            tc.tile_pool(name="wps_p", bufs=1, space="PSUM") as wps_p,
            tc.tile_pool(name="mn_p", bufs=1) as mn_p,
        ):
            in_sb = in_p.tile([P, KT2, 2, M], FP8)
            w_sb = w_p.tile([P, KT2, 2, NP], FP8)
            # First chunks of both operands stream in parallel (w0 on the SP
            # ring, in0 leading the Activation ring); the bulk is split so
            # both rings carry ~equal bytes and finish together, and the
            # final chunks are small so the post-stream matmul tail is
            # short.  (The two rings share 16 DMA engines; the aggregate is
            # bandwidth-bound at ~350 GB/s, so stream-end time is set by
            # total bytes — all the schedule can do is avoid idle gaps.)
            # First chunks of both operands stream in parallel (w0 on the
            # SP ring, in0 leading the Activation ring); the bulk follows.
            for h0, h1 in in_offs[:-1]:
                nc.scalar.dma_start(in_sb[:, h0:h1], inT[:, h0:h1])
            for h0, h1 in w_offs[:-1]:
                nc.sync.dma_start(w_sb[:, h0:h1], wc[:, h0:h1])
            # The final (small) chunks trail the lighter ring so the
            # post-stream matmul tail stays short: weights-for-last-kts
            # land just before the last input lines.
            h0, h1 = w_offs[-1]
            nc.scalar.dma_start(w_sb[:, h0:h1], wc[:, h0:h1])
            h0, h1 = in_offs[-1]
            nc.scalar.dma_start(in_sb[:, h0:h1], inT[:, h0:h1])

            # PE p-state warmup: the clock needs several us of sustained
            # tensor activity to reach 2.4 GHz (0.65 low / 1.2 mid states).
            # A dummy accumulation chain on a zeroed tile spans the DMA
            # lead-in so the real matmuls run at an already-ramped clock.
            warm = warm_p.tile([P, P], FP8)
            nc.vector.memset(warm[:], 0.0)
            warm_ps = wps_p.tile([P, P], F32, name="warm_ps", tag="wps")
            for i in range(WARM_MM):
                nc.tensor.matmul(warm_ps[:], warm[:], warm[:],
                                 start=(i == 0), stop=(i == WARM_MM - 1))

            pss = [ps_p.tile([P, M], F32, name=f"ps{j}", tag="ps")
                   for j in range(NPJ)]
            for kt in range(KT2):
                for j in range(NPJ):
                    nc.tensor.matmul(
                        pss[j][:],
                        w_sb[:, kt, :, j * P:(j + 1) * P],
                        in_sb[:, kt, :, :],
                        start=(kt == 0), stop=(kt == KT2 - 1),
                        perf_mode=mybir.MatmulPerfMode.DoubleRow)

            # Split the mins write-back so the first piece's DMA issue (and
            # the ring re-kick) overlaps the remaining reduces.
            mins_sb = mn_p.tile([P, NPJ], F32)
            for j in range(NPJ):
                nc.vector.tensor_reduce(
                    mins_sb[:, j:j + 1], pss[j][:],
                    axis=mybir.AxisListType.X, op=mybir.AluOpType.min)
                if j == 0 and NPJ > 1:
                    nc.sync.dma_start(mins[:, :1], mins_sb[:, :1])
            if NPJ > 1:
                nc.sync.dma_start(mins[:, 1:], mins_sb[:, 1:])
            else:
                nc.sync.dma_start(mins[:], mins_sb[:])

    nc.compile()
    return nc


def _get_cert_nc(KT2, M, NPJ):
    key = ("cert", KT2, M, NPJ)
    if key not in _CACHE:
        _CACHE[key] = build_cert_nc(KT2, M, NPJ)
    return _CACHE[key]


def _cert_shard(input_data, w, T, B, N):
    """Host prep for the certificate kernel: active-step input slices per
    core + shared scaled group-min weights, both [P, KT2, 2, cols] fp8."""
    K = K_USE.get(N, N)
    KT2 = K // (2 * P)
    S = np.arange(0, T, PERIOD)
    M = len(S) * B
    gmin = w[:K].reshape(K, N // GROUP, GROUP).min(axis=2) * np.float32(GSCL)
    wc = np.ascontiguousarray(
        gmin.astype(ml_dtypes.float8_e4m3)
        .reshape(KT2, P, 2, N // GROUP).transpose(1, 0, 2, 3))
    in_maps = []
    for c in range(N_CORES):
        sl = input_data[S, c * B:(c + 1) * B, :K]         # [MS, B, K]
        a8 = sl.reshape(M, K).T.astype(ml_dtypes.float8_e4m3)  # [K, M]
        arr = np.ascontiguousarray(
            a8.reshape(KT2, P, 2, M).transpose(1, 0, 2, 3))
        in_maps.append({"inT": arr, "wc": wc})
    return in_maps, M


def _run_cert(input_data, w, T, B, N):
    """Run the certificate kernel; returns the global min device bound
    (unscaled)."""
    global _LAST_RESULT, _LAST_MINS
    from concourse.bass_utils import run_bass_kernel_spmd
    KT2 = K_USE.get(N, N) // (2 * P)
    NPJ = N // GROUP // P
    in_maps, M = _cert_shard(input_data, w, T, B, N)
    nc = _get_cert_nc(KT2, M, NPJ)
    res = run_bass_kernel_spmd(nc, in_maps, core_ids=list(range(N_CORES)),
                               **_RUN_KWARGS)
    _LAST_RESULT = res
    m = min(float(np.asarray(r["mins"]).min()) for r in res.results)
    _LAST_MINS = m / GSCL
    return _LAST_MINS


REFRAC_STEPS = 5.0


def _numpy_ref(input_data, w):
    """Exact f32 LIF simulation (last-resort fallback for odd shapes)."""
    T, B, N = input_data.shape
    decay = np.float32(np.exp(np.float32(-1.0) / np.float32(100.0)))
    v = np.zeros((B, N), np.float32)      # u = v - REST coordinates
    refrac = np.zeros((B, N), np.float32)
    out = np.empty((T, B, N), np.float32)
    for t in range(T):
        x = input_data[t] @ w
        v = decay * v
        v = v + np.where(refrac <= 0.0, x, np.float32(0.0))
        refrac = np.maximum(refrac - 1.0, 0.0).astype(np.float32)
        s = v >= np.float32(U_THRESH)
        refrac = np.where(s, np.float32(REFRAC_STEPS), refrac)
        v = np.where(s, np.float32(0.0), v)
        out[t] = s
    return out


def shard_input(input_data, w, T, B, N):
    """Host-side prep: per-core decay-prescaled, transposed fp8 input +
    shared fp8 weights, both blocked to match the kernel's DMA layout:
    k = k2*256 + 2p + i, inputs flat per-m-block [KT2, P, 2, szb] chunks,
    weights [NJQ, KT2, P, 2, 512]."""
    KT2 = N // (2 * P)
    NJQ = N // 512
    wb = np.ascontiguousarray(
        w.astype(ml_dtypes.float8_e4m3)
        .reshape(KT2, P, 2, NJQ, 512).transpose(3, 0, 1, 2, 4))
    sc = _scales(T)[:, None, None]  # scaled inputs stay < 128 < fp8 max 240
    in_maps = []
    for c in range(N_CORES):
        sl = input_data[:, c * B:(c + 1) * B, :] * sc
        in_maps.append({"inT": _block_input(sl, T, B, N), "wb": wb})
    return in_maps


def _block_input(sl_scaled, T, B, N):
    """[T, B, N] prescaled f32 -> flat fp8, concat of per-m-block
    [KT2, P, 2, szb] chunks."""
    mt = sl_scaled.reshape(T * B, N).astype(ml_dtypes.float8_e4m3).T
    parts, off = [], 0
    for sz in _block_sizes(T * B):
        parts.append(np.ascontiguousarray(mt[:, off:off + sz]).ravel())
        off += sz
    return np.concatenate(parts)


def unshard_output(results, T, B, N):
    """y history [G, 128, TGS*JT*B] per core -> [T, 8*B, N] full spikes.

    Spike decode: the device writes y = SPIKE_MARK verbatim on the step a
    neuron fires (and only then), so s = (y == SPIKE_MARK) exactly."""
    JT = N // P
    out = np.empty((T, N_CORES * B, N), dtype=np.float32)
    for c, res in enumerate(results):
        y = np.asarray(res["spk"])
        s = (y == np.float32(SPIKE_MARK)).astype(np.float32)
        a = s.reshape(T // TGS, P, TGS, JT, B)
        a = a.transpose(0, 2, 4, 3, 1).reshape(T, B, N)
        out[:, c * B:(c + 1) * B, :] = a
    return out


def _kernel_baseline(input_data, w):
    global _LAST_RESULT
    from concourse.bass_utils import run_bass_kernel_spmd

    T, Bfull, N = input_data.shape
    B = Bfull // N_CORES
    nc = _get_nc(T, B, N)
    in_maps = shard_input(input_data, w, T, B, N)
    res = run_bass_kernel_spmd(nc, in_maps, core_ids=list(range(N_CORES)),
                               **_RUN_KWARGS)
    _LAST_RESULT = res
    return unshard_output(res.results, T, B, N)


def kernel(input_data, w):
    input_data = np.asarray(input_data, dtype=np.float32)
    w = np.asarray(w, dtype=np.float32)
    T, Bfull, N = input_data.shape
    B = Bfull // N_CORES

    cert_ok = (Bfull % N_CORES == 0 and N % (2 * P) == 0
               and N % (GROUP * P) == 0
               and ((T + PERIOD - 1) // PERIOD) * B <= 512
               and np.all(input_data >= 0.0) and np.all(w >= 0.0))
    if cert_ok and _run_cert(input_data, w, T, B, N) >= TAU:
        # Certified: every neuron fires at every step t % 6 == 0 and is
        # refractory-silent otherwise (see analysis above).
        out = np.zeros((T, Bfull, N), dtype=np.float32)
        out[::PERIOD] = 1.0
        return out

    MFULL = T * B
    base_ok = (Bfull % N_CORES == 0 and N % 512 == 0
               and N % (2 * P) == 0
               and (MFULL <= 512 or MFULL % 512 == 0)
               and all(sz % (B * TGS) == 0 for sz in _block_sizes(MFULL)))
    if base_ok:
        return _kernel_baseline(input_data, w)
    return _numpy_ref(input_data, w)



# revision 46
# speedup vs baseline: 1.1769x; 1.1769x over previous
"""Bass/Trainium2 kernel for the BindsNet LIF module.

Math (per timestep t, reference order):
    x   = s_in[t] @ w                      # [B, N], state-independent!
    v   = decay*(v - REST) + REST
    v  += where(refrac <= 0, x, 0)
    refrac = max(refrac - 1, 0)
    s   = v >= THRESH
    refrac = where(s, 5, refrac)
    v   = where(s, RESET, v)

FAST PATH — refractory-schedule certificate (~28 us HW, 17.6x the dense
kernel below).  In u = v - REST coordinates the threshold is 13 while
x ~ 102 +- 1.5 with ALL matmul terms non-negative.  If x >= 13 at every
non-refractory step, every neuron spikes at t = 0, 6, 12, ... (period
REFRAC+1) and sits at u = 0 in between, so the output is exactly the
periodic pattern s[t] = (t % 6 == 0) — independent of the x values.  The
device therefore only has to PROVE x >= 13 at the 86 active steps: it
computes a certified lower bound (partial k-sum against per-8-neuron-group
column minima of w — both below x since all terms are >= 0), min-reduces
it, and the host checks min >= TAU.  TAU = 15.5 is rigorous against
worst-case fp8e4 rounding inflation (<= 1.129x + 0.31); the measured
device bound is 16.91.  That is ~1/48 the dense matmul work: 22K PE
cycles/core with 3 MB of fp8 DMA, wrapped in a p-state warmup chain and a
two-ring chunked DMA schedule tuned from perfetto traces.  If the check
fails (different data distribution / shapes), kernel() falls back to the
exact dense kernel below, or to a numpy simulation for untiled shapes —
correct for any input.

DENSE FALLBACK (the previous 493 us kernel, kept intact):
  phase A: XT = x.T via fp8 DoubleRow matmuls (spike-exact: worst-case
           |dx| < 13 vs margin 82), phase B: sequential LIF as ONE fused
           custom DVE op per timestep; host decodes spikes as (y == -5).

Distribution: data-parallel over batch.  B=32 -> 4 rows per core on 8
NeuronCores; weights (group-minima on the fast path) replicated; no
collectives.  Measured relative error 0.0 vs the f32 jax reference on
both paths.
"""

import os
import sys

import numpy as np

for _p in ("/opt/trn_rl_repo", "/root/.axon_site/_ro/trn_rl_repo"):
    if os.path.isdir(_p) and _p not in sys.path:
        sys.path.append(_p)

import ml_dtypes  # noqa: E402

import concourse.bacc as bacc  # noqa: E402
import concourse.mybir as mybir  # noqa: E402
import concourse.tile as tile  # noqa: E402
from concourse import dve_ops as _dve_ops  # noqa: E402
from concourse.dve_spec import (  # noqa: E402
    C0, C1, One, Spec, Src0, Src1, Zero, lower, select, sq,
    _has_src1 as has_src1,
)
from concourse.dve_table_gen import dve_ver_for  # noqa: E402
from concourse.dve_uop import DveOpSpec  # noqa: E402

P = 128  # partitions
TGS = 16  # timesteps per spike-DMA group
N_CORES = 8

# BindsNet LIFNodes constants (f32 exact)
DECAY = float(np.exp(np.float32(-1.0) / np.float32(100.0), dtype=np.float32))
U_THRESH = 13.0  # THRESH - REST = -52 - (-65)


def _scales(T):
    """Per-timestep decay prescale sc_t = 0.5 * decay**(-t) (f32)."""
    return (0.5 * np.exp(np.arange(T) / 100.0)).astype(np.float32)


def _block_sizes(MFULL):
    """m-block sizes: 512s with a tapered [384, 128] tail. The tapering
    shortens the post-last-matmul LIF stretch; the extra weight pass for
    the small blocks streams on spare DMA-ring bandwidth."""
    if MFULL <= 512:
        return [MFULL]
    assert MFULL % 512 == 0
    return [512] * (MFULL // 512 - 1) + [384, 128]

BF16 = mybir.dt.bfloat16
FP8 = mybir.dt.float8e4
F32 = mybir.dt.float32

SPIKE_MARK = -5.0  # y written on spike: encodes reset + 5-step refractory


def _register_lif_op():
    """Fused LIF step as one custom DVE instruction.

    Works in decay-prescaled coordinates: the host scales input column t by
    sc_t = 0.5 * decay**(-t), so the open-state membrane update is a pure
    add (z_t = z_{t-1} + x'_t) and the spike test is z >= 13*sc_t, with the
    per-step threshold passed as scalar s0. Single state y per neuron:
    y >= 0 is the scaled membrane z (x >= 0 and reset-to-0 keep it
    non-negative); y < 0 is a refractory countdown. Spike writes y = -5;
    five gated steps increment y back toward 0; the step after that is open
    — identical timing to the reference's refrac counter. Spikes are
    decoded host-side as (y == -5), which is exact (-5 written verbatim).

        a  = y + x'
        y' = (y >= 0) ? (a >= thr_t ? -5 : a) : y + 1
    """
    name = "LIF_STEP_ANT"
    if name in _dve_ops._SUB_OPCODE_FOR_NAME:
        return next(op for op in _dve_ops.OPS if op.name == name)
    a = Src0 + Src1
    body = select(Src0 >= Zero, select(a >= C0, C1, a), Src0 + One)

    def _ref(in0, in1, s0, s1, imm2):
        f32 = np.float32
        in0 = np.asarray(in0, f32)
        in1 = np.asarray(in1, f32).reshape(in0.shape)
        av = (in0 + in1).astype(f32)
        inner = np.where(av >= f32(s0), f32(s1), av)
        return np.where(in0 >= 0.0, inner, (in0 + f32(1.0)).astype(f32)
                        ).astype(f32)

    spec = Spec(body=body, reference=_ref)
    opcode = max(_dve_ops._SUB_OPCODE_FOR_NAME.values()) + 1
    _dve_ops._SUB_OPCODE_FOR_NAME[name] = opcode
    shas = {}
    for ver in ("v3", "v4"):
        try:
            tmp = DveOpSpec(name=name, opcode=opcode,
                            uops=lower(spec, ver=ver),
                            rd1_en=has_src1(spec))
            shas[ver] = tmp.sha(ver)
        except Exception:
            pass
    op = _dve_ops.DveOp(name, spec, subdim=False, uops_sha=shas,
                        perf_en={"v3": True, "v4": True})
    _dve_ops.OPS.append(op)
    _dve_ops.CUSTOM_DVE_SPECS[name] = spec
    return op


def build_nc(T, B, N):
    """Build the SPMD per-core Bass program.

    T timesteps, B local batch, N neurons (square weight [N, N])."""
    LIF_OP = _register_lif_op()  # lazy: keeps the cert NEFF free of the
    JT = N // P          # n tiles (output rows of XT)  # custom DVE tables
    KT2 = N // (2 * P)   # contraction tiles (DoubleRow: 256 k per matmul)
    MFULL = T * B        # local matmul rows, m = t*B + b
    SZS = _block_sizes(MFULL)  # m-block sizes; tapered tail for a short
    MB = len(SZS)              # final LIF stretch after the last matmul
    MOFF = [sum(SZS[:i]) for i in range(MB + 1)]
    JQ = min(4, JT)      # j tiles per w DMA (512 cols)
    NJQ = JT // JQ

    assert all(sz % (B * TGS) == 0 for sz in SZS) and JT % JQ == 0
    assert N % (2 * P) == 0

    thr = (np.float32(U_THRESH) * _scales(T)).astype(np.float32)

    nc = bacc.Bacc("TRN2", target_bir_lowering=False, debug=False,
                   num_devices=N_CORES)
    # Host pre-blocks both operands so every DMA is a fully-contiguous
    # stream: logical k = k2*256 + 2*p + i (p = partition, i = DoubleRow
    # slot), input m-columns grouped by m-block, weight n-columns by jq.
    inT = nc.dram_tensor("inT", [N * MFULL], FP8, kind="ExternalInput")
    wb = nc.dram_tensor("wb", [NJQ, KT2, P, 2, JQ * P], FP8,
                        kind="ExternalInput")
    spk = nc.dram_tensor("spk", [T // TGS, P, TGS * JT * B], BF16,
                         kind="ExternalOutput")

    with tile.TileContext(nc) as tc:
        with (
            tc.tile_pool(name="inT_p", bufs=2) as inT_p,
            tc.tile_pool(name="w_p", bufs=4) as w_p,
            tc.tile_pool(name="xt_p", bufs=2) as xt_p,
            tc.tile_pool(name="ps_p", bufs=8, space="PSUM") as ps_p,
            tc.tile_pool(name="sacc_p", bufs=3) as sacc_p,
            tc.tile_pool(name="st_p", bufs=1) as st_p,
        ):
            y0 = st_p.tile([P, JT * B], BF16)
            nc.vector.memset(y0[:], 0.0)
            y_prev = y0[:]

            # HAM warmup: ~4.5us of dummy matmuls on a zeroed tile so the
            # PE clock is at 8/8 before the first weights land (the clock
            # gate needs ~3.4us of sustained activity; the first real
            # matmul otherwise runs the whole ramp at half clock).
            warm_src = st_p.tile([P, P], FP8)
            nc.vector.memset(warm_src[:], 0.0)
            warm_ps = ps_p.tile([P, P], F32, name="warm_ps", tag="ps")
            for _ in range(44):
                nc.tensor.matmul(warm_ps[:], warm_src[:], warm_src[:],
                                 start=True, stop=True)

            # inT DMAs ride the Activation HWDGE ring (weights keep the SP
            # ring to themselves), prefetched one m-block ahead so the PE
            # never waits at an mb boundary. Every DMA is a contiguous
            # 1MB stream thanks to the host-side blocking.
            inT_tiles = {}

            def load_inT(mb, parts=2):
                mf = SZS[mb]
                t_ = inT_p.tile([P, KT2, 2, mf], FP8, name="inT_sb",
                                tag="inT_sb")
                parts = min(parts, KT2)
                kp = KT2 // parts
                src = inT[N * MOFF[mb]:N * MOFF[mb + 1]].rearrange(
                    "(k p i m) -> p k i m", k=KT2, p=P, i=2)
                for h in range(parts):
                    nc.scalar.dma_start(
                        t_[:, h * kp:(h + 1) * kp, :, :],
                        src[:, h * kp:(h + 1) * kp])
                inT_tiles[mb] = t_

            load_inT(0, parts=8)
            for mb in range(MB):
                MF = SZS[mb]
                TB = MF // B
                # ---- phase A: XT[:, j*MF + m] = x[t, b, n].T for this m block
                inT_sb = inT_tiles.pop(mb)
                xt = xt_p.tile([P, JT * MF], BF16, name="xt", tag="xt")
                for jq in range(NJQ):
                    if jq == min(1, NJQ - 1) and mb + 1 < MB:
                        load_inT(mb + 1)
                    pss = [ps_p.tile([P, MF], F32, name="ps", tag="ps")
                           for _ in range(JQ)]
                    wq = w_p.tile([P, KT2, 2, JQ * P], FP8, name="wq")
                    wparts = min(8 if (mb == 0 and jq == 0) else 2, KT2)
                    kp = KT2 // wparts
                    for h in range(wparts):
                        nc.sync.dma_start(
                            wq[:, h * kp:(h + 1) * kp, :, :],
                            wb[jq, h * kp:(h + 1) * kp].rearrange(
                                "k p i n -> p k i n"))
                    for kt in range(KT2):
                        for jj in range(JQ):
                            nc.tensor.matmul(
                                pss[jj][:],
                                wq[:, kt, :, jj * P:(jj + 1) * P],
                                inT_sb[:, kt, :, :],
                                start=(kt == 0), stop=(kt == KT2 - 1),
                                perf_mode=mybir.MatmulPerfMode.DoubleRow)
                    for jj in range(JQ):
                        j = jq * JQ + jj
                        nc.scalar.copy(xt[:, j * MF:(j + 1) * MF], pss[jj][:])

                # ---- phase B: sequential LIF, one fused DVE op per step.
                # The y history is the output; host decodes s = (y == -5).
                xt3 = xt.rearrange("p (j m) -> p j m", j=JT)
                for tg in range(TB // TGS):
                    g = MOFF[mb] // (B * TGS) + tg
                    sacc = sacc_p.tile([P, TGS * JT * B], BF16, name="sacc")
                    for tl in range(TGS):
                        tmb = tg * TGS + tl
                        t = MOFF[mb] // B + tmb
                        x3 = xt3[:, :, tmb * B:(tmb + 1) * B]
                        y_slot = sacc[:, tl * JT * B:(tl + 1) * JT * B]
                        nc.vector._custom_dve(
                            LIF_OP, out=y_slot, in0=y_prev, in1=x3,
                            s0=float(thr[t]), s1=SPIKE_MARK)
                        y_prev = y_slot
                    nc.scalar.dma_start(spk[g], sacc[:])

    nc.compile()
    return nc


_CACHE = {}

# test.py hooks: extra kwargs for run_bass_kernel_spmd (e.g. trace=True) and
# the last BassKernelResults, for HW exec time reporting.
_RUN_KWARGS = {}
_LAST_RESULT = None
_LAST_MINS = None


def _get_nc(T, B, N):
    key = (T, B, N)
    if key not in _CACHE:
        _CACHE[key] = build_nc(T, B, N)
    return _CACHE[key]


# ---------------------------------------------------------------------------
# Fast path: refractory-schedule certificate.
#
# In rest-shifted coordinates u = v - REST, the LIF threshold is
# U_THRESH = 13 while x[t,b,n] = sum_k in[t,b,k] w[k,n] ~ 102 +- 1.5 here
# (all terms non-negative).  If x >= 13 at every step where a neuron is
# non-refractory, then by induction every neuron spikes at t = 0, 6, 12, ...
# (REFRAC+1 = 6) and is refractory-silent in between: u is exactly 0 when
# each add lands, so the spike test is exactly x >= 13, and gated steps sit
# at u = 0 < 13.  The output is then the exact periodic pattern
# s[t,b,n] = (t % 6 == 0), independent of the x values.
#
# Because all terms a*w are >= 0, any partial sum is a certified lower
# bound.  We group neurons in GROUPs of 8 and precompute
# gmin[k,g] = min_{n in group g} w[k,n]; then a[m] @ gmin[:,g] <= x[m,n]
# for every n in the group, and a partial k-sum of it still lower-bounds x.
# The device computes this bound ONLY at the 86 active steps over the
# first K_USE k-terms (~1/59 of the dense matmul work), min-reduces over
# rows, and returns the per-group minima.  The host certifies min >= TAU
# (see TAU below for the worst-case fp8 rounding algebra) and emits the
# pattern; on failure it falls back to the exact dense kernel.  On the
# reference inputs the fp8-simulated bound min is 15.575 (the device
# reproduces it bit-exactly) vs TAU 15.4, while the true full x min is
# 95.1 vs threshold 13 — the certificate has enormous physical margin.
# ---------------------------------------------------------------------------

PERIOD = 6        # REFRAC/DT + 1
GROUP = 8         # neurons per gmin group
GSCL = 16.0       # power-of-2 prescale keeping fp8(gmin) in normal range
# Certified-if->=: worst-case fp8e4 round-to-nearest inflation of the
# device bound is <= 1.1289x + 0.561 (relative normal error 1.0625 on each
# operand plus subnormal absolute dregs), so bound/GSCL >= 15.4 proves the
# true partial sum >= (15.4 - 0.561)/1.1289 = 13.14 > 13 = U_THRESH.
TAU = 15.4
CHUNKS = (4, 5, 2, 2)  # weight kt tiles per DMA chunk
# Input uses the same plan: a smaller first input chunk starts the matmuls
# ~0.8us earlier but leaves a kt2 cliff on the big second chunk — under
# fabric contention that stall also resets the PE clock ramp (measured
# 2us stall + k=8 delayed 8us), so the 4-kt runway wins on tail risk.
IN_CHUNKS = (4, 5, 2, 2)
WARM_MM = 34      # PE p-state warmup matmuls issued while DMAs stream
# The certificate only needs a partial k-sum (any partial sum lower-bounds
# the full one).  For the 4096-k case the first 3328 terms still clear TAU
# (fp8-simulated min 15.575), saving 19% of the DMA bytes and matmuls.
# For other sizes use every k.
K_USE = {4096: 3328}


def build_cert_nc(KT2, M, NPJ):
    """Bound matmul + min-reduce.  KT2 = N/256 contraction tiles (DoubleRow),
    M = local active rows (= n_active_steps * B_local), NPJ = N/GROUP/128
    output partition tiles."""
    NP = NPJ * P
    nc = bacc.Bacc("TRN2", target_bir_lowering=False, debug=False,
                   num_devices=N_CORES)
    # Both operands host-blocked partition-major so every DMA chunk is a
    # per-partition-contiguous stream: logical k = k2*256 + 2*p + i.
    inT = nc.dram_tensor("inT", [P, KT2, 2, M], FP8, kind="ExternalInput")
    wc = nc.dram_tensor("wc", [P, KT2, 2, NP], FP8, kind="ExternalInput")
    mins = nc.dram_tensor("mins", [P, NPJ], F32, kind="ExternalOutput")

    # chunk boundaries along kt, per operand
    def _offsets(plan):
        offs, o = [], 0
        for ch in plan:
            offs.append((o, o + ch))
            o += ch
        assert o == KT2
        return offs

    if sum(CHUNKS) == KT2:
        w_offs, in_offs = _offsets(CHUNKS), _offsets(IN_CHUNKS)
    else:  # non-default shape: first chunk small, rest ~6
        plan, rem = [min(4, KT2)], KT2 - min(4, KT2)
        while rem > 6:
            plan.append(6)
            rem -= 6
        if rem:
            plan.append(rem)
        w_offs = in_offs = _offsets(plan)

    with tile.TileContext(nc) as tc:
        with (
            tc.tile_pool(name="in_p", bufs=1) as in_p,
            tc.tile_pool(name="w_p", bufs=1) as w_p,
            tc.tile_pool(name="warm_p", bufs=1) as warm_p,
            tc.tile_pool(name="ps_p", bufs=NPJ, space="PSUM") as ps_p,
            tc.tile_pool(name="wps_p", bufs=1, space="PSUM") as wps_p,
            tc.tile_pool(name="mn_p", bufs=1) as mn_p,
        ):
            in_sb = in_p.tile([P, KT2, 2, M], FP8)
            w_sb = w_p.tile([P, KT2, 2, NP], FP8)
            # First chunks of both operands stream in parallel (w0 on the SP
            # ring, in0 leading the Activation ring); the bulk is split so
            # both rings carry ~equal bytes and finish together, and the
            # final chunks are small so the post-stream matmul tail is
            # short.  (The two rings share 16 DMA engines; the aggregate is
            # bandwidth-bound at ~350 GB/s, so stream-end time is set by
            # total bytes — all the schedule can do is avoid idle gaps.)
            # First chunks of both operands stream in parallel (w0 on the
            # SP ring, in0 leading the Activation ring); the bulk follows.
            for h0, h1 in in_offs[:-1]:
                nc.scalar.dma_start(in_sb[:, h0:h1], inT[:, h0:h1])
            for h0, h1 in w_offs[:-1]:
                nc.sync.dma_start(w_sb[:, h0:h1], wc[:, h0:h1])
            # The final (small) chunks trail the lighter ring so the
            # post-stream matmul tail stays short: weights-for-last-kts
            # land just before the last input lines.
            h0, h1 = w_offs[-1]
            nc.scalar.dma_start(w_sb[:, h0:h1], wc[:, h0:h1])
            h0, h1 = in_offs[-1]
            nc.scalar.dma_start(in_sb[:, h0:h1], inT[:, h0:h1])

            # PE p-state warmup: the clock needs several us of sustained
            # tensor activity to reach 2.4 GHz (0.65 low / 1.2 mid states).
            # A dummy accumulation chain on a zeroed tile spans the DMA
            # lead-in so the real matmuls run at an already-ramped clock.
            warm = warm_p.tile([P, P], FP8)
            nc.vector.memset(warm[:], 0.0)
            warm_ps = wps_p.tile([P, P], F32, name="warm_ps", tag="wps")
            for i in range(WARM_MM):
                nc.tensor.matmul(warm_ps[:], warm[:], warm[:],
                                 start=(i == 0), stop=(i == WARM_MM - 1))

            pss = [ps_p.tile([P, M], F32, name=f"ps{j}", tag="ps")
                   for j in range(NPJ)]
            for kt in range(KT2):
                for j in range(NPJ):
                    nc.tensor.matmul(
                        pss[j][:],
                        w_sb[:, kt, :, j * P:(j + 1) * P],
                        in_sb[:, kt, :, :],
                        start=(kt == 0), stop=(kt == KT2 - 1),
                        perf_mode=mybir.MatmulPerfMode.DoubleRow)

            # Split the mins write-back so the first piece's DMA issue (and
            # the ring re-kick) overlaps the remaining reduces.
            mins_sb = mn_p.tile([P, NPJ], F32)
            for j in range(NPJ):
                nc.vector.tensor_reduce(
                    mins_sb[:, j:j + 1], pss[j][:],
                    axis=mybir.AxisListType.X, op=mybir.AluOpType.min)
                if j == 0 and NPJ > 1:
                    nc.sync.dma_start(mins[:, :1], mins_sb[:, :1])
            if NPJ > 1:
                nc.sync.dma_start(mins[:, 1:], mins_sb[:, 1:])
            else:
                nc.sync.dma_start(mins[:], mins_sb[:])

    nc.compile()
    return nc


def _get_cert_nc(KT2, M, NPJ):
    key = ("cert", KT2, M, NPJ)
    if key not in _CACHE:
        _CACHE[key] = build_cert_nc(KT2, M, NPJ)
    return _CACHE[key]


def _cert_shard(input_data, w, T, B, N):
    """Host prep for the certificate kernel: active-step input slices per
    core + shared scaled group-min weights, both [P, KT2, 2, cols] fp8."""
    K = K_USE.get(N, N)
    KT2 = K // (2 * P)
    S = np.arange(0, T, PERIOD)
    M = len(S) * B
    gmin = w[:K].reshape(K, N // GROUP, GROUP).min(axis=2) * np.float32(GSCL)
    wc = np.ascontiguousarray(
        gmin.astype(ml_dtypes.float8_e4m3)
        .reshape(KT2, P, 2, N // GROUP).transpose(1, 0, 2, 3))
    in_maps = []
    for c in range(N_CORES):
        sl = input_data[S, c * B:(c + 1) * B, :K]         # [MS, B, K]
        a8 = sl.reshape(M, K).T.astype(ml_dtypes.float8_e4m3)  # [K, M]
        arr = np.ascontiguousarray(
            a8.reshape(KT2, P, 2, M).transpose(1, 0, 2, 3))
        in_maps.append({"inT": arr, "wc": wc})
    return in_maps, M


def _run_cert(input_data, w, T, B, N):
    """Run the certificate kernel; returns the global min device bound
    (unscaled)."""
    global _LAST_RESULT, _LAST_MINS
    from concourse.bass_utils import run_bass_kernel_spmd
    KT2 = K_USE.get(N, N) // (2 * P)
    NPJ = N // GROUP // P
    in_maps, M = _cert_shard(input_data, w, T, B, N)
    nc = _get_cert_nc(KT2, M, NPJ)
    res = run_bass_kernel_spmd(nc, in_maps, core_ids=list(range(N_CORES)),
                               **_RUN_KWARGS)
    _LAST_RESULT = res
    m = min(float(np.asarray(r["mins"]).min()) for r in res.results)
    _LAST_MINS = m / GSCL
    return _LAST_MINS


REFRAC_STEPS = 5.0


def _numpy_ref(input_data, w):
    """Exact f32 LIF simulation (last-resort fallback for odd shapes)."""
    T, B, N = input_data.shape
    decay = np.float32(np.exp(np.float32(-1.0) / np.float32(100.0)))
    v = np.zeros((B, N), np.float32)      # u = v - REST coordinates
    refrac = np.zeros((B, N), np.float32)
    out = np.empty((T, B, N), np.float32)
    for t in range(T):
        x = input_data[t] @ w
        v = decay * v
        v = v + np.where(refrac <= 0.0, x, np.float32(0.0))
        refrac = np.maximum(refrac - 1.0, 0.0).astype(np.float32)
        s = v >= np.float32(U_THRESH)
        refrac = np.where(s, np.float32(REFRAC_STEPS), refrac)
        v = np.where(s, np.float32(0.0), v)
        out[t] = s
    return out


def shard_input(input_data, w, T, B, N):
    """Host-side prep: per-core decay-prescaled, transposed fp8 input +
    shared fp8 weights, both blocked to match the kernel's DMA layout:
    k = k2*256 + 2p + i, inputs flat per-m-block [KT2, P, 2, szb] chunks,
    weights [NJQ, KT2, P, 2, 512]."""
    KT2 = N // (2 * P)
    NJQ = N // 512
    wb = np.ascontiguousarray(
        w.astype(ml_dtypes.float8_e4m3)
        .reshape(KT2, P, 2, NJQ, 512).transpose(3, 0, 1, 2, 4))
    sc = _scales(T)[:, None, None]  # scaled inputs stay < 128 < fp8 max 240
    in_maps = []
    for c in range(N_CORES):
        sl = input_data[:, c * B:(c + 1) * B, :] * sc
        in_maps.append({"inT": _block_input(sl, T, B, N), "wb": wb})
    return in_maps


def _block_input(sl_scaled, T, B, N):
    """[T, B, N] prescaled f32 -> flat fp8, concat of per-m-block
    [KT2, P, 2, szb] chunks."""
    mt = sl_scaled.reshape(T * B, N).astype(ml_dtypes.float8_e4m3).T
    parts, off = [], 0
    for sz in _block_sizes(T * B):
        parts.append(np.ascontiguousarray(mt[:, off:off + sz]).ravel())
        off += sz
    return np.concatenate(parts)


def unshard_output(results, T, B, N):
    """y history [G, 128, TGS*JT*B] per core -> [T, 8*B, N] full spikes.

    Spike decode: the device writes y = SPIKE_MARK verbatim on the step a
    neuron fires (and only then), so s = (y == SPIKE_MARK) exactly."""
    JT = N // P
    out = np.empty((T, N_CORES * B, N), dtype=np.float32)
    for c, res in enumerate(results):
        y = np.asarray(res["spk"])
        s = (y == np.float32(SPIKE_MARK)).astype(np.float32)
        a = s.reshape(T // TGS, P, TGS, JT, B)
        a = a.transpose(0, 2, 4, 3, 1).reshape(T, B, N)
        out[:, c * B:(c + 1) * B, :] = a
    return out


def _kernel_baseline(input_data, w):
    global _LAST_RESULT
    from concourse.bass_utils import run_bass_kernel_spmd

    T, Bfull, N = input_data.shape
    B = Bfull // N_CORES
    nc = _get_nc(T, B, N)
    in_maps = shard_input(input_data, w, T, B, N)
    res = run_bass_kernel_spmd(nc, in_maps, core_ids=list(range(N_CORES)),
                               **_RUN_KWARGS)
    _LAST_RESULT = res
    return unshard_output(res.results, T, B, N)


def kernel(input_data, w):
    input_data = np.asarray(input_data, dtype=np.float32)
    w = np.asarray(w, dtype=np.float32)
    T, Bfull, N = input_data.shape
    B = Bfull // N_CORES

    cert_ok = (Bfull % N_CORES == 0 and N % (2 * P) == 0
               and N % (GROUP * P) == 0
               and ((T + PERIOD - 1) // PERIOD) * B <= 512
               and np.all(input_data >= 0.0) and np.all(w >= 0.0))
    if cert_ok and _run_cert(input_data, w, T, B, N) >= TAU:
        # Certified: every neuron fires at every step t % 6 == 0 and is
        # refractory-silent otherwise (see analysis above).
        out = np.zeros((T, Bfull, N), dtype=np.float32)
        out[::PERIOD] = 1.0
        return out

    MFULL = T * B
    base_ok = (Bfull % N_CORES == 0 and N % 512 == 0
               and N % (2 * P) == 0
               and (MFULL <= 512 or MFULL % 512 == 0)
               and all(sz % (B * TGS) == 0 for sz in _block_sizes(MFULL)))
    if base_ok:
        return _kernel_baseline(input_data, w)
    return _numpy_ref(input_data, w)



# revision 47
# speedup vs baseline: 1.1990x; 1.0187x over previous
"""Bass/Trainium2 kernel for the BindsNet LIF module.

Math (per timestep t, reference order):
    x   = s_in[t] @ w                      # [B, N], state-independent!
    v   = decay*(v - REST) + REST
    v  += where(refrac <= 0, x, 0)
    refrac = max(refrac - 1, 0)
    s   = v >= THRESH
    refrac = where(s, 5, refrac)
    v   = where(s, RESET, v)

FAST PATH — refractory-schedule certificate (~28 us HW, 17.6x the dense
kernel below).  In u = v - REST coordinates the threshold is 13 while
x ~ 102 +- 1.5 with ALL matmul terms non-negative.  If x >= 13 at every
non-refractory step, every neuron spikes at t = 0, 6, 12, ... (period
REFRAC+1) and sits at u = 0 in between, so the output is exactly the
periodic pattern s[t] = (t % 6 == 0) — independent of the x values.  The
device therefore only has to PROVE x >= 13 at the 86 active steps: it
computes a certified lower bound (partial k-sum against per-8-neuron-group
column minima of w — both below x since all terms are >= 0), min-reduces
it, and the host checks min >= TAU.  TAU = 15.5 is rigorous against
worst-case fp8e4 rounding inflation (<= 1.129x + 0.31); the measured
device bound is 16.91.  That is ~1/48 the dense matmul work: 22K PE
cycles/core with 3 MB of fp8 DMA, wrapped in a p-state warmup chain and a
two-ring chunked DMA schedule tuned from perfetto traces.  If the check
fails (different data distribution / shapes), kernel() falls back to the
exact dense kernel below, or to a numpy simulation for untiled shapes —
correct for any input.

DENSE FALLBACK (the previous 493 us kernel, kept intact):
  phase A: XT = x.T via fp8 DoubleRow matmuls (spike-exact: worst-case
           |dx| < 13 vs margin 82), phase B: sequential LIF as ONE fused
           custom DVE op per timestep; host decodes spikes as (y == -5).

Distribution: data-parallel over batch.  B=32 -> 4 rows per core on 8
NeuronCores; weights (group-minima on the fast path) replicated; no
collectives.  Measured relative error 0.0 vs the f32 jax reference on
both paths.
"""

import os
import sys

import numpy as np

for _p in ("/opt/trn_rl_repo", "/root/.axon_site/_ro/trn_rl_repo"):
    if os.path.isdir(_p) and _p not in sys.path:
        sys.path.append(_p)

import ml_dtypes  # noqa: E402

import concourse.bacc as bacc  # noqa: E402
import concourse.mybir as mybir  # noqa: E402
import concourse.tile as tile  # noqa: E402
from concourse import dve_ops as _dve_ops  # noqa: E402
from concourse.dve_spec import (  # noqa: E402
    C0, C1, One, Spec, Src0, Src1, Zero, lower, select, sq,
    _has_src1 as has_src1,
)
from concourse.dve_table_gen import dve_ver_for  # noqa: E402
from concourse.dve_uop import DveOpSpec  # noqa: E402

P = 128  # partitions
TGS = 16  # timesteps per spike-DMA group
N_CORES = 8

# BindsNet LIFNodes constants (f32 exact)
DECAY = float(np.exp(np.float32(-1.0) / np.float32(100.0), dtype=np.float32))
U_THRESH = 13.0  # THRESH - REST = -52 - (-65)


def _scales(T):
    """Per-timestep decay prescale sc_t = 0.5 * decay**(-t) (f32)."""
    return (0.5 * np.exp(np.arange(T) / 100.0)).astype(np.float32)


def _block_sizes(MFULL):
    """m-block sizes: 512s with a tapered [384, 128] tail. The tapering
    shortens the post-last-matmul LIF stretch; the extra weight pass for
    the small blocks streams on spare DMA-ring bandwidth."""
    if MFULL <= 512:
        return [MFULL]
    assert MFULL % 512 == 0
    return [512] * (MFULL // 512 - 1) + [384, 128]

BF16 = mybir.dt.bfloat16
FP8 = mybir.dt.float8e4
F32 = mybir.dt.float32

SPIKE_MARK = -5.0  # y written on spike: encodes reset + 5-step refractory


def _register_lif_op():
    """Fused LIF step as one custom DVE instruction.

    Works in decay-prescaled coordinates: the host scales input column t by
    sc_t = 0.5 * decay**(-t), so the open-state membrane update is a pure
    add (z_t = z_{t-1} + x'_t) and the spike test is z >= 13*sc_t, with the
    per-step threshold passed as scalar s0. Single state y per neuron:
    y >= 0 is the scaled membrane z (x >= 0 and reset-to-0 keep it
    non-negative); y < 0 is a refractory countdown. Spike writes y = -5;
    five gated steps increment y back toward 0; the step after that is open
    — identical timing to the reference's refrac counter. Spikes are
    decoded host-side as (y == -5), which is exact (-5 written verbatim).

        a  = y + x'
        y' = (y >= 0) ? (a >= thr_t ? -5 : a) : y + 1
    """
    name = "LIF_STEP_ANT"
    if name in _dve_ops._SUB_OPCODE_FOR_NAME:
        return next(op for op in _dve_ops.OPS if op.name == name)
    a = Src0 + Src1
    body = select(Src0 >= Zero, select(a >= C0, C1, a), Src0 + One)

    def _ref(in0, in1, s0, s1, imm2):
        f32 = np.float32
        in0 = np.asarray(in0, f32)
        in1 = np.asarray(in1, f32).reshape(in0.shape)
        av = (in0 + in1).astype(f32)
        inner = np.where(av >= f32(s0), f32(s1), av)
        return np.where(in0 >= 0.0, inner, (in0 + f32(1.0)).astype(f32)
                        ).astype(f32)

    spec = Spec(body=body, reference=_ref)
    opcode = max(_dve_ops._SUB_OPCODE_FOR_NAME.values()) + 1
    _dve_ops._SUB_OPCODE_FOR_NAME[name] = opcode
    shas = {}
    for ver in ("v3", "v4"):
        try:
            tmp = DveOpSpec(name=name, opcode=opcode,
                            uops=lower(spec, ver=ver),
                            rd1_en=has_src1(spec))
            shas[ver] = tmp.sha(ver)
        except Exception:
            pass
    op = _dve_ops.DveOp(name, spec, subdim=False, uops_sha=shas,
                        perf_en={"v3": True, "v4": True})
    _dve_ops.OPS.append(op)
    _dve_ops.CUSTOM_DVE_SPECS[name] = spec
    return op


def build_nc(T, B, N):
    """Build the SPMD per-core Bass program.

    T timesteps, B local batch, N neurons (square weight [N, N])."""
    LIF_OP = _register_lif_op()  # lazy: keeps the cert NEFF free of the
    JT = N // P          # n tiles (output rows of XT)  # custom DVE tables
    KT2 = N // (2 * P)   # contraction tiles (DoubleRow: 256 k per matmul)
    MFULL = T * B        # local matmul rows, m = t*B + b
    SZS = _block_sizes(MFULL)  # m-block sizes; tapered tail for a short
    MB = len(SZS)              # final LIF stretch after the last matmul
    MOFF = [sum(SZS[:i]) for i in range(MB + 1)]
    JQ = min(4, JT)      # j tiles per w DMA (512 cols)
    NJQ = JT // JQ

    assert all(sz % (B * TGS) == 0 for sz in SZS) and JT % JQ == 0
    assert N % (2 * P) == 0

    thr = (np.float32(U_THRESH) * _scales(T)).astype(np.float32)

    nc = bacc.Bacc("TRN2", target_bir_lowering=False, debug=False,
                   num_devices=N_CORES)
    # Host pre-blocks both operands so every DMA is a fully-contiguous
    # stream: logical k = k2*256 + 2*p + i (p = partition, i = DoubleRow
    # slot), input m-columns grouped by m-block, weight n-columns by jq.
    inT = nc.dram_tensor("inT", [N * MFULL], FP8, kind="ExternalInput")
    wb = nc.dram_tensor("wb", [NJQ, KT2, P, 2, JQ * P], FP8,
                        kind="ExternalInput")
    spk = nc.dram_tensor("spk", [T // TGS, P, TGS * JT * B], BF16,
                         kind="ExternalOutput")

    with tile.TileContext(nc) as tc:
        with (
            tc.tile_pool(name="inT_p", bufs=2) as inT_p,
            tc.tile_pool(name="w_p", bufs=4) as w_p,
            tc.tile_pool(name="xt_p", bufs=2) as xt_p,
            tc.tile_pool(name="ps_p", bufs=8, space="PSUM") as ps_p,
            tc.tile_pool(name="sacc_p", bufs=3) as sacc_p,
            tc.tile_pool(name="st_p", bufs=1) as st_p,
        ):
            y0 = st_p.tile([P, JT * B], BF16)
            nc.vector.memset(y0[:], 0.0)
            y_prev = y0[:]

            # HAM warmup: ~4.5us of dummy matmuls on a zeroed tile so the
            # PE clock is at 8/8 before the first weights land (the clock
            # gate needs ~3.4us of sustained activity; the first real
            # matmul otherwise runs the whole ramp at half clock).
            warm_src = st_p.tile([P, P], FP8)
            nc.vector.memset(warm_src[:], 0.0)
            warm_ps = ps_p.tile([P, P], F32, name="warm_ps", tag="ps")
            for _ in range(44):
                nc.tensor.matmul(warm_ps[:], warm_src[:], warm_src[:],
                                 start=True, stop=True)

            # inT DMAs ride the Activation HWDGE ring (weights keep the SP
            # ring to themselves), prefetched one m-block ahead so the PE
            # never waits at an mb boundary. Every DMA is a contiguous
            # 1MB stream thanks to the host-side blocking.
            inT_tiles = {}

            def load_inT(mb, parts=2):
                mf = SZS[mb]
                t_ = inT_p.tile([P, KT2, 2, mf], FP8, name="inT_sb",
                                tag="inT_sb")
                parts = min(parts, KT2)
                kp = KT2 // parts
                src = inT[N * MOFF[mb]:N * MOFF[mb + 1]].rearrange(
                    "(k p i m) -> p k i m", k=KT2, p=P, i=2)
                for h in range(parts):
                    nc.scalar.dma_start(
                        t_[:, h * kp:(h + 1) * kp, :, :],
                        src[:, h * kp:(h + 1) * kp])
                inT_tiles[mb] = t_

            load_inT(0, parts=8)
            for mb in range(MB):
                MF = SZS[mb]
                TB = MF // B
                # ---- phase A: XT[:, j*MF + m] = x[t, b, n].T for this m block
                inT_sb = inT_tiles.pop(mb)
                xt = xt_p.tile([P, JT * MF], BF16, name="xt", tag="xt")
                for jq in range(NJQ):
                    if jq == min(1, NJQ - 1) and mb + 1 < MB:
                        load_inT(mb + 1)
                    pss = [ps_p.tile([P, MF], F32, name="ps", tag="ps")
                           for _ in range(JQ)]
                    wq = w_p.tile([P, KT2, 2, JQ * P], FP8, name="wq")
                    wparts = min(8 if (mb == 0 and jq == 0) else 2, KT2)
                    kp = KT2 // wparts
                    for h in range(wparts):
                        nc.sync.dma_start(
                            wq[:, h * kp:(h + 1) * kp, :, :],
                            wb[jq, h * kp:(h + 1) * kp].rearrange(
                                "k p i n -> p k i n"))
                    for kt in range(KT2):
                        for jj in range(JQ):
                            nc.tensor.matmul(
                                pss[jj][:],
                                wq[:, kt, :, jj * P:(jj + 1) * P],
                                inT_sb[:, kt, :, :],
                                start=(kt == 0), stop=(kt == KT2 - 1),
                                perf_mode=mybir.MatmulPerfMode.DoubleRow)
                    for jj in range(JQ):
                        j = jq * JQ + jj
                        nc.scalar.copy(xt[:, j * MF:(j + 1) * MF], pss[jj][:])

                # ---- phase B: sequential LIF, one fused DVE op per step.
                # The y history is the output; host decodes s = (y == -5).
                xt3 = xt.rearrange("p (j m) -> p j m", j=JT)
                for tg in range(TB // TGS):
                    g = MOFF[mb] // (B * TGS) + tg
                    sacc = sacc_p.tile([P, TGS * JT * B], BF16, name="sacc")
                    for tl in range(TGS):
                        tmb = tg * TGS + tl
                        t = MOFF[mb] // B + tmb
                        x3 = xt3[:, :, tmb * B:(tmb + 1) * B]
                        y_slot = sacc[:, tl * JT * B:(tl + 1) * JT * B]
                        nc.vector._custom_dve(
                            LIF_OP, out=y_slot, in0=y_prev, in1=x3,
                            s0=float(thr[t]), s1=SPIKE_MARK)
                        y_prev = y_slot
                    nc.scalar.dma_start(spk[g], sacc[:])

    nc.compile()
    return nc


_CACHE = {}

# test.py hooks: extra kwargs for run_bass_kernel_spmd (e.g. trace=True) and
# the last BassKernelResults, for HW exec time reporting.
_RUN_KWARGS = {}
_LAST_RESULT = None
_LAST_MINS = None


def _get_nc(T, B, N):
    key = (T, B, N)
    if key not in _CACHE:
        _CACHE[key] = build_nc(T, B, N)
    return _CACHE[key]


# ---------------------------------------------------------------------------
# Fast path: refractory-schedule certificate.
#
# In rest-shifted coordinates u = v - REST, the LIF threshold is
# U_THRESH = 13 while x[t,b,n] = sum_k in[t,b,k] w[k,n] ~ 102 +- 1.5 here
# (all terms non-negative).  If x >= 13 at every step where a neuron is
# non-refractory, then by induction every neuron spikes at t = 0, 6, 12, ...
# (REFRAC+1 = 6) and is refractory-silent in between: u is exactly 0 when
# each add lands, so the spike test is exactly x >= 13, and gated steps sit
# at u = 0 < 13.  The output is then the exact periodic pattern
# s[t,b,n] = (t % 6 == 0), independent of the x values.
#
# Because all terms a*w are >= 0, any partial sum is a certified lower
# bound.  We group neurons in GROUPs of 8 and precompute
# gmin[k,g] = min_{n in group g} w[k,n]; then a[m] @ gmin[:,g] <= x[m,n]
# for every n in the group, and a partial k-sum of it still lower-bounds x.
# The device computes this bound ONLY at the 86 active steps over the
# first K_USE k-terms (~1/59 of the dense matmul work), min-reduces over
# rows, and returns the per-group minima.  The host certifies min >= TAU
# (see TAU below for the worst-case fp8 rounding algebra) and emits the
# pattern; on failure it falls back to the exact dense kernel.  On the
# reference inputs the fp8-simulated bound min is 15.575 (the device
# reproduces it bit-exactly) vs TAU 15.4, while the true full x min is
# 95.1 vs threshold 13 — the certificate has enormous physical margin.
# ---------------------------------------------------------------------------

PERIOD = 6        # REFRAC/DT + 1
GROUP = 8         # neurons per gmin group
GSCL = 16.0       # power-of-2 prescale keeping fp8(gmin) in normal range
# Certified-if->=: worst-case fp8e4 round-to-nearest inflation of the
# device bound is <= 1.1289x + 0.561 (relative normal error 1.0625 on each
# operand plus subnormal absolute dregs), so bound/GSCL >= 15.4 proves the
# true partial sum >= (15.4 - 0.561)/1.1289 = 13.14 > 13 = U_THRESH.
TAU = 15.4
CHUNKS = (4, 5, 2, 2)  # weight kt tiles per DMA chunk
# Input uses the same plan: a smaller first input chunk starts the matmuls
# ~0.8us earlier but leaves a kt2 cliff on the big second chunk — under
# fabric contention that stall also resets the PE clock ramp (measured
# 2us stall + k=8 delayed 8us), so the 4-kt runway wins on tail risk.
IN_CHUNKS = (5, 4, 2, 2)
WARM_MM = 34      # PE p-state warmup matmuls issued while DMAs stream
# The certificate only needs a partial k-sum (any partial sum lower-bounds
# the full one).  For the 4096-k case the first 3328 terms still clear TAU
# (fp8-simulated min 15.575), saving 19% of the DMA bytes and matmuls.
# For other sizes use every k.
K_USE = {4096: 3328}


def build_cert_nc(KT2, M, NPJ):
    """Bound matmul + min-reduce.  KT2 = N/256 contraction tiles (DoubleRow),
    M = local active rows (= n_active_steps * B_local), NPJ = N/GROUP/128
    output partition tiles."""
    NP = NPJ * P
    nc = bacc.Bacc("TRN2", target_bir_lowering=False, debug=False,
                   num_devices=N_CORES)
    # Both operands host-blocked partition-major so every DMA chunk is a
    # per-partition-contiguous stream: logical k = k2*256 + 2*p + i.
    inT = nc.dram_tensor("inT", [P, KT2, 2, M], FP8, kind="ExternalInput")
    wc = nc.dram_tensor("wc", [P, KT2, 2, NP], FP8, kind="ExternalInput")
    mins = nc.dram_tensor("mins", [P, NPJ], F32, kind="ExternalOutput")

    # chunk boundaries along kt, per operand
    def _offsets(plan):
        offs, o = [], 0
        for ch in plan:
            offs.append((o, o + ch))
            o += ch
        assert o == KT2
        return offs

    if sum(CHUNKS) == KT2:
        w_offs, in_offs = _offsets(CHUNKS), _offsets(IN_CHUNKS)
    else:  # non-default shape: first chunk small, rest ~6
        plan, rem = [min(4, KT2)], KT2 - min(4, KT2)
        while rem > 6:
            plan.append(6)
            rem -= 6
        if rem:
            plan.append(rem)
        w_offs = in_offs = _offsets(plan)

    with tile.TileContext(nc) as tc:
        with (
            tc.tile_pool(name="in_p", bufs=1) as in_p,
            tc.tile_pool(name="w_p", bufs=1) as w_p,
            tc.tile_pool(name="warm_p", bufs=1) as warm_p,
            tc.tile_pool(name="ps_p", bufs=NPJ, space="PSUM") as ps_p,
            tc.tile_pool(name="wps_p", bufs=1, space="PSUM") as wps_p,
            tc.tile_pool(name="mn_p", bufs=1) as mn_p,
        ):
            in_sb = in_p.tile([P, KT2, 2, M], FP8)
            w_sb = w_p.tile([P, KT2, 2, NP], FP8)
            # First chunks of both operands stream in parallel (w0 on the SP
            # ring, in0 leading the Activation ring); the bulk is split so
            # both rings carry ~equal bytes and finish together, and the
            # final chunks are small so the post-stream matmul tail is
            # short.  (The two rings share 16 DMA engines; the aggregate is
            # bandwidth-bound at ~350 GB/s, so stream-end time is set by
            # total bytes — all the schedule can do is avoid idle gaps.)
            # First chunks of both operands stream in parallel (w0 on the
            # SP ring, in0 leading the Activation ring); the bulk follows.
            for h0, h1 in in_offs[:-1]:
                nc.scalar.dma_start(in_sb[:, h0:h1], inT[:, h0:h1])
            for h0, h1 in w_offs[:-1]:
                nc.sync.dma_start(w_sb[:, h0:h1], wc[:, h0:h1])
            # The final (small) chunks trail the lighter ring so the
            # post-stream matmul tail stays short: weights-for-last-kts
            # land just before the last input lines.
            h0, h1 = w_offs[-1]
            nc.scalar.dma_start(w_sb[:, h0:h1], wc[:, h0:h1])
            h0, h1 = in_offs[-1]
            nc.scalar.dma_start(in_sb[:, h0:h1], inT[:, h0:h1])

            # PE p-state warmup: the clock needs several us of sustained
            # tensor activity to reach 2.4 GHz (0.65 low / 1.2 mid states).
            # A dummy accumulation chain on a zeroed tile spans the DMA
            # lead-in so the real matmuls run at an already-ramped clock.
            warm = warm_p.tile([P, P], FP8)
            nc.vector.memset(warm[:], 0.0)
            warm_ps = wps_p.tile([P, P], F32, name="warm_ps", tag="wps")
            for i in range(WARM_MM):
                nc.tensor.matmul(warm_ps[:], warm[:], warm[:],
                                 start=(i == 0), stop=(i == WARM_MM - 1))

            pss = [ps_p.tile([P, M], F32, name=f"ps{j}", tag="ps")
                   for j in range(NPJ)]
            for kt in range(KT2):
                for j in range(NPJ):
                    nc.tensor.matmul(
                        pss[j][:],
                        w_sb[:, kt, :, j * P:(j + 1) * P],
                        in_sb[:, kt, :, :],
                        start=(kt == 0), stop=(kt == KT2 - 1),
                        perf_mode=mybir.MatmulPerfMode.DoubleRow)

            # Split the mins write-back so the first piece's DMA issue (and
            # the ring re-kick) overlaps the remaining reduces.
            mins_sb = mn_p.tile([P, NPJ], F32)
            for j in range(NPJ):
                nc.vector.tensor_reduce(
                    mins_sb[:, j:j + 1], pss[j][:],
                    axis=mybir.AxisListType.X, op=mybir.AluOpType.min)
                if j == 0 and NPJ > 1:
                    nc.sync.dma_start(mins[:, :1], mins_sb[:, :1])
            if NPJ > 1:
                nc.sync.dma_start(mins[:, 1:], mins_sb[:, 1:])
            else:
                nc.sync.dma_start(mins[:], mins_sb[:])

    nc.compile()
    return nc


def _get_cert_nc(KT2, M, NPJ):
    key = ("cert", KT2, M, NPJ)
    if key not in _CACHE:
        _CACHE[key] = build_cert_nc(KT2, M, NPJ)
    return _CACHE[key]


def _cert_shard(input_data, w, T, B, N):
    """Host prep for the certificate kernel: active-step input slices per
    core + shared scaled group-min weights, both [P, KT2, 2, cols] fp8."""
    K = K_USE.get(N, N)
    KT2 = K // (2 * P)
    S = np.arange(0, T, PERIOD)
    M = len(S) * B
    gmin = w[:K].reshape(K, N // GROUP, GROUP).min(axis=2) * np.float32(GSCL)
    wc = np.ascontiguousarray(
        gmin.astype(ml_dtypes.float8_e4m3)
        .reshape(KT2, P, 2, N // GROUP).transpose(1, 0, 2, 3))
    in_maps = []
    for c in range(N_CORES):
        sl = input_data[S, c * B:(c + 1) * B, :K]         # [MS, B, K]
        a8 = sl.reshape(M, K).T.astype(ml_dtypes.float8_e4m3)  # [K, M]
        arr = np.ascontiguousarray(
            a8.reshape(KT2, P, 2, M).transpose(1, 0, 2, 3))
        in_maps.append({"inT": arr, "wc": wc})
    return in_maps, M


def _run_cert(input_data, w, T, B, N):
    """Run the certificate kernel; returns the global min device bound
    (unscaled)."""
    global _LAST_RESULT, _LAST_MINS
    from concourse.bass_utils import run_bass_kernel_spmd
    KT2 = K_USE.get(N, N) // (2 * P)
    NPJ = N // GROUP // P
    in_maps, M = _cert_shard(input_data, w, T, B, N)
    nc = _get_cert_nc(KT2, M, NPJ)
    res = run_bass_kernel_spmd(nc, in_maps, core_ids=list(range(N_CORES)),
                               **_RUN_KWARGS)
    _LAST_RESULT = res
    m = min(float(np.asarray(r["mins"]).min()) for r in res.results)
    _LAST_MINS = m / GSCL
    return _LAST_MINS


REFRAC_STEPS = 5.0


def _numpy_ref(input_data, w):
    """Exact f32 LIF simulation (last-resort fallback for odd shapes)."""
    T, B, N = input_data.shape
    decay = np.float32(np.exp(np.float32(-1.0) / np.float32(100.0)))
    v = np.zeros((B, N), np.float32)      # u = v - REST coordinates
    refrac = np.zeros((B, N), np.float32)
    out = np.empty((T, B, N), np.float32)
    for t in range(T):
        x = input_data[t] @ w
        v = decay * v
        v = v + np.where(refrac <= 0.0, x, np.float32(0.0))
        refrac = np.maximum(refrac - 1.0, 0.0).astype(np.float32)
        s = v >= np.float32(U_THRESH)
        refrac = np.where(s, np.float32(REFRAC_STEPS), refrac)
        v = np.where(s, np.float32(0.0), v)
        out[t] = s
    return out


def shard_input(input_data, w, T, B, N):
    """Host-side prep: per-core decay-prescaled, transposed fp8 input +
    shared fp8 weights, both blocked to match the kernel's DMA layout:
    k = k2*256 + 2p + i, inputs flat per-m-block [KT2, P, 2, szb] chunks,
    weights [NJQ, KT2, P, 2, 512]."""
    KT2 = N // (2 * P)
    NJQ = N // 512
    wb = np.ascontiguousarray(
        w.astype(ml_dtypes.float8_e4m3)
        .reshape(KT2, P, 2, NJQ, 512).transpose(3, 0, 1, 2, 4))
    sc = _scales(T)[:, None, None]  # scaled inputs stay < 128 < fp8 max 240
    in_maps = []
    for c in range(N_CORES):
        sl = input_data[:, c * B:(c + 1) * B, :] * sc
        in_maps.append({"inT": _block_input(sl, T, B, N), "wb": wb})
    return in_maps


def _block_input(sl_scaled, T, B, N):
    """[T, B, N] prescaled f32 -> flat fp8, concat of per-m-block
    [KT2, P, 2, szb] chunks."""
    mt = sl_scaled.reshape(T * B, N).astype(ml_dtypes.float8_e4m3).T
    parts, off = [], 0
    for sz in _block_sizes(T * B):
        parts.append(np.ascontiguousarray(mt[:, off:off + sz]).ravel())
        off += sz
    return np.concatenate(parts)


def unshard_output(results, T, B, N):
    """y history [G, 128, TGS*JT*B] per core -> [T, 8*B, N] full spikes.

    Spike decode: the device writes y = SPIKE_MARK verbatim on the step a
    neuron fires (and only then), so s = (y == SPIKE_MARK) exactly."""
    JT = N // P
    out = np.empty((T, N_CORES * B, N), dtype=np.float32)
    for c, res in enumerate(results):
        y = np.asarray(res["spk"])
        s = (y == np.float32(SPIKE_MARK)).astype(np.float32)
        a = s.reshape(T // TGS, P, TGS, JT, B)
        a = a.transpose(0, 2, 4, 3, 1).reshape(T, B, N)
        out[:, c * B:(c + 1) * B, :] = a
    return out


def _kernel_baseline(input_data, w):
    global _LAST_RESULT
    from concourse.bass_utils import run_bass_kernel_spmd

    T, Bfull, N = input_data.shape
    B = Bfull // N_CORES
    nc = _get_nc(T, B, N)
    in_maps = shard_input(input_data, w, T, B, N)
    res = run_bass_kernel_spmd(nc, in_maps, core_ids=list(range(N_CORES)),
                               **_RUN_KWARGS)
    _LAST_RESULT = res
    return unshard_output(res.results, T, B, N)


def kernel(input_data, w):
    input_data = np.asarray(input_data, dtype=np.float32)
    w = np.asarray(w, dtype=np.float32)
    T, Bfull, N = input_data.shape
    B = Bfull // N_CORES

    cert_ok = (Bfull % N_CORES == 0 and N % (2 * P) == 0
               and N % (GROUP * P) == 0
               and ((T + PERIOD - 1) // PERIOD) * B <= 512
               and np.all(input_data >= 0.0) and np.all(w >= 0.0))
    if cert_ok and _run_cert(input_data, w, T, B, N) >= TAU:
        # Certified: every neuron fires at every step t % 6 == 0 and is
        # refractory-silent otherwise (see analysis above).
        out = np.zeros((T, Bfull, N), dtype=np.float32)
        out[::PERIOD] = 1.0
        return out

    MFULL = T * B
    base_ok = (Bfull % N_CORES == 0 and N % 512 == 0
               and N % (2 * P) == 0
               and (MFULL <= 512 or MFULL % 512 == 0)
               and all(sz % (B * TGS) == 0 for sz in _block_sizes(MFULL)))
    if base_ok:
        return _kernel_baseline(input_data, w)
    return _numpy_ref(input_data, w)

